# revision 11
# baseline (speedup 1.0000x reference)
"""CvT attention block (depthwise-conv projections + MHA) on 8 TRN2 NeuronCores.

Strategy: pure data-parallel over batch (B=32 -> 4 images per core, no
collectives). Per core, everything is computed in channel-major layout on
chip:

  - x [4,785,384] arrives bf16 (host pre-cast); spatial tokens are
    DMA-transposed straight into a zero-padded channel-major buffer
    xpad[c, b, 30, 30].
  - depthwise 3x3 conv (+folded BN) runs as 9 scalar_tensor_tensor FMAs on
    the vector engine (per-partition weight scalars), stride 1 for q,
    stride 2 for k/v. cls token is passed through.
  - Q/K/V projections are weight-stationary bf16 matmuls; V is produced
    token-major [t, heads*(64 V | 64 ones)] so the attention O-matmul also
    emits the softmax denominator broadcast across 64 partitions.
  - S^T = K_h^T Q_h per (image, head); exp (with 1/sqrt(384) scale) fused
    into the PSUM->SBUF move on the scalar engine; no max-subtraction
    (logits are ~1e-2 for this problem scale).
  - O = [V|ones]^T P^T gives rows 0:64 = unnormalized output, rows 64:128 =
    denominator; reciprocal + tensor_mul fuse normalization into the
    PSUM->SBUF move.
  - final projection is token-major (lhsT = O channel-major chunks), bias
    added via a K=1 matmul row, single big strided DMA out.

Token order on chip is [spatial(784) | cls] per image so the conv output is
written at aligned offsets; the output DMA un-permutes.
"""

import numpy as np

C = 384
T = 785
TKV = 197
BPC = 4  # batch per core
NCORES = 8
SCALE = float(C) ** -0.5
BN_EPS = 1e-5

_STATE = {}


def _build(debug=False):
    import sys
    if "/opt/trn_rl_repo" not in sys.path:
        sys.path.insert(0, "/opt/trn_rl_repo")
    import concourse.bass as bass
    import concourse.mybir as mybir
    from concourse import bacc
    import concourse.tile as tile

    f32 = mybir.dt.float32
    bf16 = mybir.dt.bfloat16
    Exp = mybir.ActivationFunctionType.Exp
    mult = mybir.AluOpType.mult
    add = mybir.AluOpType.add

    nc = bacc.Bacc("TRN2", target_bir_lowering=False, debug=False, num_devices=NCORES)

    x_d = nc.dram_tensor("x", [BPC, T, C], bf16, kind="ExternalInput")
    wq_d = nc.dram_tensor("wq", [C, C], bf16, kind="ExternalInput")  # w_q.T [cin, cout]
    wk_d = nc.dram_tensor("wk", [C, C], bf16, kind="ExternalInput")
    wv_d = nc.dram_tensor("wv", [C, C], bf16, kind="ExternalInput")
    wp_d = nc.dram_tensor("wp", [C, C], bf16, kind="ExternalInput")
    cw_d = nc.dram_tensor("cw", [3, C, 9], f32, kind="ExternalInput")  # BN-folded dw conv
    cb_d = nc.dram_tensor("cb", [3, C, 1], f32, kind="ExternalInput")  # BN-folded bias
    bp_d = nc.dram_tensor("bp", [1, C], bf16, kind="ExternalInput")  # b_proj
    out_d = nc.dram_tensor("out", [BPC, T, C], f32, kind="ExternalOutput")
    out_flat = out_d.ap().rearrange("b t c -> (b t) c")
    if debug:
        dbg = {
            "dxpad": nc.dram_tensor("dxpad", [3, 128, BPC, 30, 30], bf16, kind="ExternalOutput"),
            "dqc": nc.dram_tensor("dqc", [3, 128, BPC, 786], bf16, kind="ExternalOutput"),
            "dkc": nc.dram_tensor("dkc", [3, 128, BPC, 198], bf16, kind="ExternalOutput"),
            "dvc": nc.dram_tensor("dvc", [3, 128, BPC, 198], bf16, kind="ExternalOutput"),
            "dQcm": nc.dram_tensor("dQcm", [3, 128, BPC, T], bf16, kind="ExternalOutput"),
            "dKcm": nc.dram_tensor("dKcm", [3, 128, BPC, TKV], bf16, kind="ExternalOutput"),
            "dVE0": nc.dram_tensor("dVE0", [BPC, 128, 6, 128], bf16, kind="ExternalOutput"),
            "dVE1": nc.dram_tensor("dVE1", [BPC, 69, 6, 128], bf16, kind="ExternalOutput"),
            "dOcm": nc.dram_tensor("dOcm", [3, 128, BPC, 786], bf16, kind="ExternalOutput"),
        }

    with tile.TileContext(nc) as tc:
        with tc.tile_pool(name="statics", bufs=1) as st:
            # ---- static SBUF buffers ----
            wq_s = [st.tile([128, C], bf16, name=f"wq{i}") for i in range(3)]
            wk_s = [st.tile([128, C], bf16, name=f"wk{i}") for i in range(3)]
            wv_s = [st.tile([128, C], bf16, name=f"wv{i}") for i in range(3)]
            wp_s = [st.tile([128, C], bf16, name=f"wp{i}") for i in range(3)]
            cw_s = [[st.tile([128, 9], f32, name=f"cw{p}_{i}") for i in range(3)] for p in range(3)]
            cb_s = [[st.tile([128, 1], f32, name=f"cb{p}_{i}") for i in range(3)] for p in range(3)]
            bp_s = st.tile([1, C], bf16, name="bp")
            ones_s = st.tile([1, 128], bf16, name="ones")
            xpad = [st.tile([128, BPC, 30, 30], bf16, name=f"xpad{i}") for i in range(3)]
            xcm = [st.tile([128, BPC, 784], bf16, name=f"xcm{i}") for i in range(3)]
            xcls = [st.tile([128, BPC], bf16, name=f"xcls{i}") for i in range(3)]
            # conv outputs, token order [spatial | cls | pad]
            qc = [st.tile([128, BPC, 786], bf16, name=f"qc{i}") for i in range(3)]
            kc = [st.tile([128, BPC, 198], bf16, name=f"kc{i}") for i in range(3)]
            vc = [st.tile([128, BPC, 198], bf16, name=f"vc{i}") for i in range(3)]
            # projected activations
            Qcm = [st.tile([128, BPC, T], bf16, name=f"Qcm{i}") for i in range(3)]
            Kcm = [st.tile([128, BPC, TKV], bf16, name=f"Kcm{i}") for i in range(3)]
            VE0 = [st.tile([128, 6, 128], bf16, name=f"VE0_{b}") for b in range(BPC)]
            VE1 = [st.tile([69, 6, 128], bf16, name=f"VE1_{b}") for b in range(BPC)]
            Ocm = [st.tile([128, BPC, 786], bf16, name=f"Ocm{i}") for i in range(3)]
            y_all = st.tile([128, BPC, 6, C], f32, name="y_all")
            y_tail = st.tile([17, BPC, C], f32, name="y_tail")

            # ---- loads ----
            for i in range(3):
                cs = slice(i * 128, (i + 1) * 128)
                nc.sync.dma_start(out=wq_s[i][:], in_=wq_d.ap()[cs, :])
                nc.sync.dma_start(out=wk_s[i][:], in_=wk_d.ap()[cs, :])
                nc.sync.dma_start(out=wv_s[i][:], in_=wv_d.ap()[cs, :])
                nc.sync.dma_start(out=wp_s[i][:], in_=wp_d.ap()[cs, :])
                for p in range(3):
                    nc.sync.dma_start(out=cw_s[p][i][:], in_=cw_d.ap()[p, cs, :])
                    nc.sync.dma_start(out=cb_s[p][i][:], in_=cb_d.ap()[p, cs, :])
            nc.sync.dma_start(out=bp_s[:], in_=bp_d.ap()[:, :])
            nc.vector.memset(ones_s[:], 1.0)

            # x: transpose spatial tokens into padded channel-major buffer
            for i in range(3):
                cs = slice(i * 128, (i + 1) * 128)
                nc.gpsimd.memset(xpad[i][:], 0.0)
                for b in range(BPC):
                    nc.sync.dma_start_transpose(
                        out=xcm[i][:, b, :], in_=x_d.ap()[b, 1:T, cs]
                    )
                    nc.vector.tensor_copy(
                        xpad[i][:, b, 1:29, 1:29],
                        xcm[i][:, b, :].rearrange("p (i j) -> p i j", i=28),
                    )
                nc.sync.dma_start_transpose(out=xcls[i][:], in_=x_d.ap()[:, 0, cs])

            # ---- depthwise conv + BN (DVE) ----
            # (out_buf, n_out_side, stride, n_tok)
            paths = [(qc, 28, 1), (kc, 14, 2), (vc, 14, 2)]
            for p, (obuf, side, stride) in enumerate(paths):
                ntok = side * side
                for i in range(3):
                    for b in range(BPC):
                        ov = obuf[i][:, b, 0:ntok].rearrange(
                            "p (i j) -> p i j", i=side)
                        for k in range(9):
                            di, dj = k // 3, k % 3
                            if stride == 1:
                                iv = xpad[i][:, b, di:di + side, dj:dj + side]
                            else:
                                iv = xpad[i][:, b, di:di + 2 * side:2, dj:dj + 2 * side:2]
                            wk_ap = cw_s[p][i][:, k:k + 1]
                            if k == 0:
                                nc.vector.tensor_scalar(
                                    out=ov, in0=iv, scalar1=wk_ap, scalar2=cb_s[p][i][:],
                                    op0=mult, op1=add,
                                )
                            else:
                                nc.vector.scalar_tensor_tensor(
                                    out=ov, in0=iv, scalar=wk_ap, in1=ov, op0=mult, op1=add,
                                )
                    # cls token passthrough
                    nc.scalar.copy(out=obuf[i][:, :, ntok], in_=xcls[i][:])

            # ---- Q/K projections (channel-major, weight stationary) ----
            with tc.tile_pool(name="ppsum", bufs=2, space="PSUM") as pp:
                for oc in range(3):
                    ocs = slice(oc * 128, (oc + 1) * 128)
                    for b in range(BPC):
                        for ts, tn in ((0, 512), (512, 273)):
                            ps = pp.tile([128, tn], f32, tag="pq")
                            for ci in range(3):
                                nc.tensor.matmul(
                                    ps[:], lhsT=wq_s[ci][:, ocs],
                                    rhs=qc[ci][:, b, ts:ts + tn],
                                    start=(ci == 0), stop=(ci == 2),
                                )
                            nc.scalar.copy(out=Qcm[oc][:, b, ts:ts + tn], in_=ps[:])
                        psk = pp.tile([128, TKV], f32, tag="pk")
                        for ci in range(3):
                            nc.tensor.matmul(
                                psk[:], lhsT=wk_s[ci][:, ocs],
                                rhs=kc[ci][:, b, 0:TKV],
                                start=(ci == 0), stop=(ci == 2),
                            )
                        nc.scalar.copy(out=Kcm[oc][:, b, :], in_=psk[:])
                # ---- V projection (token-major with interleaved ones) ----
                for b in range(BPC):
                    for sub, (ss, sn) in enumerate(((0, 128), (128, 69))):
                        psv = pp.tile([128, C], f32, tag="pv")
                        for ci in range(3):
                            nc.tensor.matmul(
                                psv[0:sn, :], lhsT=vc[ci][:, b, ss:ss + sn],
                                rhs=wv_s[ci][:],
                                start=(ci == 0), stop=(ci == 2),
                            )
                        ve = (VE0, VE1)[sub][b]
                        nc.scalar.copy(
                            out=ve[0:sn, :, 0:64],
                            in_=psv[0:sn, :].rearrange("p (h d) -> p h d", h=6),
                        )
                for b in range(BPC):
                    nc.vector.memset(VE0[b][:, :, 64:128], 1.0)
                    nc.vector.memset(VE1[b][:, :, 64:128], 1.0)

            # ---- attention per (image, head) ----
            with tc.tile_pool(name="spsum", bufs=2, space="PSUM") as sp, \
                 tc.tile_pool(name="opsum", bufs=2, space="PSUM") as op, \
                 tc.tile_pool(name="psbuf", bufs=3) as pb, \
                 tc.tile_pool(name="rbuf", bufs=2) as rbp:
                for b in range(BPC):
                    for h in range(6):
                        cc, po = h // 2, 64 * (h % 2)
                        ks = Kcm[cc][po:po + 64, b, :]
                        qs = Qcm[cc][po:po + 64, b, :]
                        s1 = sp.tile([128, T], f32, tag="s")
                        s2 = sp.tile([69, T], f32, tag="s")
                        for ls, ln in ((0, 512), (512, 273)):
                            nc.tensor.matmul(
                                s1[:, ls:ls + ln], lhsT=ks[:, 0:128],
                                rhs=qs[:, ls:ls + ln], start=True, stop=True,
                            )
                            nc.tensor.matmul(
                                s2[:, ls:ls + ln], lhsT=ks[:, 128:TKV],
                                rhs=qs[:, ls:ls + ln], start=True, stop=True,
                            )
                        p1 = pb.tile([128, T], bf16, tag="p1")
                        p2 = pb.tile([69, T], bf16, tag="p2")
                        nc.scalar.activation(out=p1[:], in_=s1[:], func=Exp, scale=SCALE)
                        nc.scalar.activation(out=p2[:], in_=s2[:], func=Exp, scale=SCALE)
                        ot = op.tile([128, T], f32, tag="o")
                        for ls, ln in ((0, 512), (512, 273)):
                            nc.tensor.matmul(
                                ot[:, ls:ls + ln], lhsT=VE0[b][:, h, :],
                                rhs=p1[:, ls:ls + ln], start=True, stop=False,
                            )
                            nc.tensor.matmul(
                                ot[:, ls:ls + ln], lhsT=VE1[b][:, h, :],
                                rhs=p2[:, ls:ls + ln], start=False, stop=True,
                            )
                        rb = rbp.tile([64, T], f32, tag="rb")
                        nc.vector.reciprocal(out=rb[:], in_=ot[64:128, :])
                        nc.vector.tensor_mul(
                            Ocm[cc][po:po + 64, b, 0:T], ot[0:64, :], rb[:]
                        )

            # ---- output projection (token-major) + bias + DMA out ----
            with tc.tile_pool(name="ypsum", bufs=4, space="PSUM") as yp:
                for b in range(BPC):
                    for ct in range(7):
                        ts, tn = ct * 128, (128 if ct < 6 else 17)
                        ypt = yp.tile([128, C], f32, tag="y")
                        for ci in range(3):
                            nc.tensor.matmul(
                                ypt[0:tn, :], lhsT=Ocm[ci][:, b, ts:ts + tn],
                                rhs=wp_s[ci][:],
                                start=(ci == 0), stop=False,
                            )
                        nc.tensor.matmul(
                            ypt[0:tn, :], lhsT=ones_s[:, 0:tn], rhs=bp_s[:],
                            start=False, stop=True,
                        )
                        if ct < 6:
                            nc.scalar.copy(out=y_all[:, b, ct, :], in_=ypt[:])
                        else:
                            nc.scalar.copy(out=y_tail[:, b, :], in_=ypt[0:17, :])

            if debug:
                for i in range(3):
                    nc.sync.dma_start(out=dbg["dxpad"].ap()[i], in_=xpad[i][:])
                    nc.sync.dma_start(out=dbg["dqc"].ap()[i], in_=qc[i][:])
                    nc.sync.dma_start(out=dbg["dkc"].ap()[i], in_=kc[i][:])
                    nc.sync.dma_start(out=dbg["dvc"].ap()[i], in_=vc[i][:])
                    nc.sync.dma_start(out=dbg["dQcm"].ap()[i], in_=Qcm[i][:])
                    nc.sync.dma_start(out=dbg["dKcm"].ap()[i], in_=Kcm[i][:])
                    nc.sync.dma_start(out=dbg["dOcm"].ap()[i], in_=Ocm[i][:])
                for b in range(BPC):
                    nc.sync.dma_start(out=dbg["dVE0"].ap()[b], in_=VE0[b][:])
                    nc.sync.dma_start(out=dbg["dVE1"].ap()[b], in_=VE1[b][:])

            import dataclasses
            for b in range(BPC):
                big_dst = dataclasses.replace(
                    out_flat,
                    offset=out_flat.offset + (b * T + 1) * C,
                    ap=[[C, 128], [128 * C, 6], [1, C]],
                )
                nc.sync.dma_start(out=big_dst, in_=y_all[:, b, :, :])
            for b in range(BPC):
                nc.sync.dma_start(
                    out=out_flat[b * T + 769:b * T + 785, :], in_=y_tail[0:16, b, :]
                )
                nc.sync.dma_start(
                    out=out_flat[b * T:b * T + 1, :], in_=y_tail[16:17, b, :]
                )

    nc.compile()
    return nc


def _prep_inputs(x, conv_w, bn_gamma, bn_beta, bn_mean, bn_var,
                 w_q, w_k, w_v, w_proj, b_proj):
    from ml_dtypes import bfloat16

    inv = (bn_gamma / np.sqrt(bn_var + BN_EPS)).astype(np.float32)  # [3,C]
    cw = (conv_w[:, :, 0, :, :].astype(np.float32)
          * inv[:, :, None, None]).reshape(3, C, 9).astype(np.float32)
    cb = (bn_beta - bn_mean * inv).astype(np.float32).reshape(3, C, 1)
    shared = {
        "wq": np.ascontiguousarray(w_q.T).astype(bfloat16),
        "wk": np.ascontiguousarray(w_k.T).astype(bfloat16),
        "wv": np.ascontiguousarray(w_v.T).astype(bfloat16),
        "wp": np.ascontiguousarray(w_proj.T).astype(bfloat16),
        "cw": cw,
        "cb": cb,
        "bp": b_proj.reshape(1, C).astype(bfloat16),
    }
    in_maps = []
    for core in range(NCORES):
        m = dict(shared)
        m["x"] = np.ascontiguousarray(
            x[core * BPC:(core + 1) * BPC]).astype(bfloat16)
        in_maps.append(m)
    return in_maps


def _run(in_maps, trace=False):
    import sys
    if "/opt/trn_rl_repo" not in sys.path:
        sys.path.insert(0, "/opt/trn_rl_repo")
    from concourse.bass_utils import run_bass_kernel_spmd

    if "nc" not in _STATE:
        _STATE["nc"] = _build()
    res = run_bass_kernel_spmd(
        _STATE["nc"], in_maps, list(range(NCORES)), trace=trace
    )
    return res


def kernel(x, conv_w, bn_gamma, bn_beta, bn_mean, bn_var,
           w_q, w_k, w_v, w_proj, b_proj, h=None, w=None, **_ignored):
    in_maps = _prep_inputs(x, conv_w, bn_gamma, bn_beta, bn_mean, bn_var,
                           w_q, w_k, w_v, w_proj, b_proj)
    res = _run(in_maps)
    out = np.concatenate(
        [res.results[i]["out"] for i in range(NCORES)], axis=0
    ).astype(np.float32)
    return out
